# revision 1
# baseline (speedup 1.0000x reference)
"""Causal self-attention (B=2, T=2048, D=2048, 16 heads) on 8 NeuronCores.

Tensor-parallel over heads: core c owns heads {2c, 2c+1}. Each core computes
its heads' Q/K/V projections, causal attention, and a partial output
projection (row-parallel); the host sums the 8 partials.

Layout strategy (fp32 storage, matmuls in fp32r at full PE rate):
  - Host passes x transposed per batch: xT [B, D, T].
  - Q^T, K^T computed as [head_dim, T] tiles (natural lhsT/rhs form for
    S^T = K Q^T); V computed as [T, head_dim].
  - Attention works on S^T [key, query] tiles, two k-tiles at a time:
    a [128,128] triangle mask is added on diagonal tiles, exp runs on ACT
    with the 1/sqrt(d) scale folded in, PV accumulates out^T [head_dim, T]
    in PSUM. On diagonal sub-tiles exp/PV/den are restricted to the valid
    column range so fully-masked columns are never computed or read.
    The softmax denominator accumulates as one wide DVE op per pair into a
    double accumulator (chain head on GpSimd); two accumulating M=1
    ones-matmuls fold + reduce it across partitions and a K=1 ones-matmul
    broadcasts the reciprocal back.
  - QKV runs interleaved with attention (block qb needs only the first
    2qb+2 token chunks) and the output projection lags one query block,
    spreading its DMA and overlapping the denominator tail.
"""
import numpy as np
import ml_dtypes
from contextlib import ExitStack

import concourse.bass as bass
import concourse.tile as tile
from concourse import bacc
from concourse import mybir
from concourse.bass_utils import run_bass_kernel_spmd

f32 = mybir.dt.float32
f32r = mybir.dt.float32r
f8 = mybir.dt.float8e4
f16 = mybir.dt.float16
E4 = ml_dtypes.float8_e4m3
DR = mybir.MatmulPerfMode.DoubleRow

B, T, D = 2, 2048, 2048
H, HD = 16, 128
N_CORES = 8
NH = H // N_CORES            # heads per core = 2
SCALE = float(HD) ** -0.5    # 1/sqrt(128)
NEG = -1.0e9

DT = D // 128                # 16 D-tiles (contraction)
DP = DT // 2                 # 8 DoubleRow di-pairs
CH = 256                     # token chunk for QKV projection
NCH = T // CH                # 8 chunks per batch
TT = T // 128                # 16 token tiles per batch
QB = 512                     # query block for attention / feature block
NQB = T // QB                # 4


def _body(ctx, tc, x8, wqkvT, woT, mask, y):
    nc = tc.nc

    singles = ctx.enter_context(tc.tile_pool(name="singles", bufs=1))
    wqkv_sb = singles.tile([128, 2, DP, 2, 3 * NH * HD], f8)
    wqkvT_r = wqkvT.rearrange("p (s a i c) -> p s a i c", s=2, a=DP, i=2)
    # wo/mask are loaded later (first needed at attention/out-proj time).
    wo_sb = singles.tile([128, NH, D], f32r)
    mask_sb = singles.tile([128, 128], f32)
    # Memset doesn't support f32r; stage in f32 and convert via DVE copy.
    ones_col_f = singles.tile([128, 1], f32)
    nc.vector.memset(ones_col_f, 1.0)
    ones_col = singles.tile([128, 1], f32r)
    nc.vector.tensor_copy(ones_col, ones_col_f)
    ones_row_f = singles.tile([1, 128], f32)
    nc.vector.memset(ones_row_f, 1.0)
    ones_row = singles.tile([1, 128], f32r)
    nc.vector.tensor_copy(ones_row, ones_row_f)
    zeros_f = singles.tile([128, 128], f32)
    nc.vector.memset(zeros_f, 0.0)
    zeros_sb = singles.tile([128, 128], f32r)
    nc.vector.tensor_copy(zeros_sb, zeros_f)

    perbatch = ctx.enter_context(tc.tile_pool(name="perbatch", bufs=1))
    xpool = ctx.enter_context(tc.tile_pool(name="xpool", bufs=2))
    attsb = ctx.enter_context(tc.tile_pool(name="attsb", bufs=4))
    densb = ctx.enter_context(tc.tile_pool(name="densb", bufs=2))
    smallsb = ctx.enter_context(tc.tile_pool(name="smallsb", bufs=2))
    ysb_pool = ctx.enter_context(tc.tile_pool(name="ysb", bufs=7))
    # PSUM budget: A{qkv acc, y} 1 bank x2 + B{s2 pairs} 2 banks x2 +
    # O{attention out} 1 bank x1 + C{den, rb} 1 bank x1 = 8 banks.
    ps = ctx.enter_context(tc.tile_pool(name="ps", bufs=2, space="PSUM"))

    for b in range(B):
        qt_sb = perbatch.tile([128, NH, T], f32r, tag="qt")    # Q^T per head
        kt_sb = perbatch.tile([128, NH, T], f32r, tag="kt")    # K^T per head
        v_sb = perbatch.tile([128, NH, TT, HD], f32r, tag="v")  # V per head
        outT_sb = perbatch.tile([128, NH, T], f32r, tag="outT")

        # QKV projection for one token chunk: six sequential accumulation
        # groups (q_h0, q_h1, k_h0, k_h1, v_tt0, v_tt1) over all 16 D-tiles.
        x8_b = x8[b].rearrange("(c p) f -> p c f", p=128)

        def qkv_chunk(ci):
            xch = xpool.tile([128, 2, DT, CH], f8, tag="xch")
            src = x8_b[:, ci, :].rearrange("p (s d t) -> p s d t", s=2, d=DT)
            if b == 0 and ci == 0:
                # Critical first loads: hi x + hi q/k weights first so the
                # first accumulation group starts early.
                nc.sync.dma_start(out=xch[:, 0, :, :], in_=src[:, 0, :, :])
                nc.sync.dma_start(out=wqkv_sb[:, 0, :, :, :],
                                  in_=wqkvT_r[:, 0, :, :, :])
                nc.sync.dma_start(out=xch[:, 1, :, :], in_=src[:, 1, :, :])
                nc.sync.dma_start(out=wqkv_sb[:, 1, :, :, :],
                                  in_=wqkvT_r[:, 1, :, :, :])
            else:
                nc.sync.dma_start(out=xch[:, 0, :, :], in_=src[:, 0, :, :])
                nc.sync.dma_start(out=xch[:, 1, :, :], in_=src[:, 1, :, :])
            groups = [("q", 0), ("q", 1), ("k", 0), ("k", 1), ("v", 0), ("v", 1)]
            # split3: hi*hi + lo*hi + hi*lo DoubleRow chains (drop lo*lo)
            chains = ((0, 0), (1, 0), (0, 1))
            for kind, idx in groups:
                acc = ps.tile([128, CH], f32, tag="A")
                n_mm = len(chains) * DP
                mm = 0
                for xs, ws in chains:
                    for dp in range(DP):
                        if kind == "q":
                            lhsT = wqkv_sb[:, ws, dp, :, idx * HD:(idx + 1) * HD]
                            rhs = xch[:, xs, 2 * dp:2 * dp + 2, :]
                        elif kind == "k":
                            lhsT = wqkv_sb[:, ws, dp, :,
                                           (NH + idx) * HD:(NH + idx + 1) * HD]
                            rhs = xch[:, xs, 2 * dp:2 * dp + 2, :]
                        else:
                            lhsT = xch[:, xs, 2 * dp:2 * dp + 2,
                                       idx * 128:(idx + 1) * 128]
                            rhs = wqkv_sb[:, ws, dp, :, 2 * NH * HD:3 * NH * HD]
                        nc.tensor.matmul(acc, lhsT=lhsT, rhs=rhs,
                                         start=(mm == 0), stop=(mm == n_mm - 1),
                                         perf_mode=DR)
                        mm += 1
                cols = slice(ci * CH, (ci + 1) * CH)
                if kind == "q":
                    nc.vector.tensor_copy(qt_sb[:, idx, cols], acc)
                elif kind == "k":
                    nc.scalar.copy(kt_sb[:, idx, cols], acc)
                else:
                    nc.vector.tensor_copy(
                        v_sb[:, :, ci * (CH // 128) + idx, :],
                        acc.rearrange("p (h d) -> p h d", h=NH))

        def out_proj(qb, half=None):
            t0 = qb * (QB // 128)
            tis = range(t0, t0 + 4) if half is None else \
                range(t0 + 2 * half, t0 + 2 * half + 2)
            for ti in tis:
                for eb in range(D // QB):
                    y_ps = ps.tile([128, QB], f32, tag="A")
                    for h in range(NH):
                        nc.tensor.matmul(
                            y_ps,
                            lhsT=outT_sb[:, h, ti * 128:(ti + 1) * 128],
                            rhs=wo_sb[:, h, eb * QB:(eb + 1) * QB],
                            start=(h == 0), stop=(h == NH - 1))
                    y_tile = ysb_pool.tile([128, QB], f16, tag="yt")
                    if (ti * (D // QB) + eb) % 8 < 3:
                        nc.vector.tensor_copy(y_tile, y_ps)
                    else:
                        nc.scalar.copy(y_tile, y_ps)
                    nc.sync.dma_start(
                        out=y[b * T + ti * 128:b * T + (ti + 1) * 128,
                              eb * QB:(eb + 1) * QB],
                        in_=y_tile)

        def attention(qb, h):
                den_eng = nc.vector
                nk = (qb + 1) * QB // 128
                o_ps = ps.tile([128, QB], f32, tag="O", bufs=1)
                # Double accumulator: one wide DVE op per k-tile pair; the
                # two halves are folded by the PE ones-matmul reduction.
                den2 = densb.tile([128, 2, QB], f32r, tag="den")
                q_slice = qt_sb[:, h, qb * QB:(qb + 1) * QB]
                for p in range(nk // 2):
                    s2 = ps.tile([128, 2, QB], f32, tag="B")
                    pt2 = attsb.tile([128, 2, QB], f32r, tag="pt")
                    for j in range(2):
                        kt = 2 * p + j
                        nc.tensor.matmul(
                            s2[:, j, :],
                            lhsT=kt_sb[:, h, kt * 128:(kt + 1) * 128],
                            rhs=q_slice, start=True, stop=True)
                    k_rel0 = (2 * p) * 128 - qb * QB
                    diag = k_rel0 >= 0
                    if diag:
                        # Diagonal pair: triangle mask, then per-subtile exp
                        # restricted to the valid column range. Columns below
                        # the diagonal are never read downstream (PV and den
                        # are restricted the same way), so no memset needed.
                        for j in range(2):
                            kr = k_rel0 + j * 128
                            nc.vector.tensor_add(
                                s2[:, j, kr:kr + 128], s2[:, j, kr:kr + 128],
                                mask_sb)
                            nc.scalar.activation(
                                pt2[:, j, kr:], s2[:, j, kr:],
                                mybir.ActivationFunctionType.Exp, scale=SCALE / 65536.0)
                    else:
                        nc.scalar.activation(
                            pt2, s2, mybir.ActivationFunctionType.Exp,
                            scale=SCALE / 65536.0)
                    for j in range(2):
                        kt = 2 * p + j
                        kr = max(k_rel0 + j * 128, 0) if diag else 0
                        nc.tensor.matmul(
                            o_ps[:, kr:], lhsT=v_sb[:, h, kt, :],
                            rhs=pt2[:, j, kr:],
                            start=(kt == 0), stop=(kt == nk - 1))
                    if p == 0:
                        if diag:
                            # qb == 0: j=0 is full width (kr=0); j=1 starts
                            # at column 128 — zero-fill the gap so the PE
                            # fold below reads initialized data.
                            nc.gpsimd.tensor_copy(den2[:, 0, :], pt2[:, 0, :])
                            nc.gpsimd.tensor_copy(den2[:, 1, 128:],
                                                  pt2[:, 1, 128:])
                            nc.vector.tensor_copy(den2[:, 1, 0:128], zeros_sb)
                        else:
                            # 1-input copy runs near line-rate on GpSimd
                            # (P12), keeping the chain head off the busy DVE.
                            nc.gpsimd.tensor_copy(den2, pt2)
                    elif diag:
                        for j in range(2):
                            kr = k_rel0 + j * 128
                            den_eng.tensor_add(den2[:, j, kr:],
                                               den2[:, j, kr:],
                                               pt2[:, j, kr:])
                    else:
                        den_eng.tensor_add(den2, den2, pt2)
                den_ps = ps.tile([1, QB], f32, tag="C", bufs=1)
                for j in range(2):
                    nc.tensor.matmul(den_ps, lhsT=ones_col, rhs=den2[:, j, :],
                                     start=(j == 0), stop=(j == 1))
                recip = smallsb.tile([1, QB], f32r, tag="rcp")
                nc.vector.reciprocal(recip, den_ps)
                rb_ps = ps.tile([128, QB], f32, tag="C", bufs=1)
                nc.tensor.matmul(rb_ps, lhsT=ones_row, rhs=recip,
                                 start=True, stop=True)
                osl = outT_sb[:, h, qb * QB:(qb + 1) * QB]
                nc.scalar.copy(osl, o_ps)
                nc.vector.tensor_mul(osl, osl, rb_ps)

        # Interleave: attention for query block qb only needs the first
        # 2*qb+2 QKV chunks, so QKV (pure PE) overlaps attention's DVE/ACT
        # load; the output projection lags one block so the denominator
        # chain of block qb overlaps block qb+1's k-loop.
        for c in range(NQB):
            qkv_chunk(2 * c)
            qkv_chunk(2 * c + 1)
            if b == 0 and c == 0:
                nc.sync.dma_start(out=mask_sb, in_=mask[:, :])
            if b == 0 and c == 1:
                nc.sync.dma_start(
                    out=wo_sb, in_=woT.rearrange("(n p) e -> p n e", p=128))
            attention(c, 0)
            if c > 0:
                out_proj(c - 1, half=0)
            attention(c, 1)
            if c > 0:
                out_proj(c - 1, half=1)
        out_proj(NQB - 1)


_NC_CACHE = {}


def build_bass(do_compile=True):
    if do_compile in _NC_CACHE:
        return _NC_CACHE[do_compile]
    nc = bacc.Bacc()
    x8 = nc.declare_dram_parameter("x8", [B, NCH * 128, 2 * DT * CH], f8,
                                   isOutput=False)
    wqkvT = nc.declare_dram_parameter("wqkvT", [128, 2 * DP * 2 * 3 * NH * HD],
                                      f8, isOutput=False)
    woT = nc.declare_dram_parameter("woT", [NH * HD, D], f32r, isOutput=False)
    mask = nc.declare_dram_parameter("mask", [128, 128], f32, isOutput=False)
    y = nc.declare_dram_parameter("y", [B * T, D], f16, isOutput=True)
    with tile.TileContext(nc) as tc:
        with ExitStack() as ctx:
            with nc.allow_low_precision(
                    reason="fp32r tiles feed full-rate PE matmuls; storage is "
                           "still 32-bit"):
                _body(ctx, tc, x8, wqkvT, woT, mask, y[:, :])
    if do_compile:
        nc.compile()
    _NC_CACHE[do_compile] = nc
    return nc


def shard_inputs(x, W_qkv, W_out):
    x = np.asarray(x, dtype=np.float32)
    W_qkv = np.asarray(W_qkv, dtype=np.float32)
    W_out = np.asarray(W_out, dtype=np.float32)

    xh = x.astype(E4)
    xl = (x - xh.astype(np.float32)).astype(E4)
    # packed per-chunk layout: [B, NCH, 128p, hi/lo, DT, CH]
    def pack_x(a):
        return a.reshape(B, NCH, CH, DT, 128).transpose(0, 1, 4, 3, 2)
    x8 = np.ascontiguousarray(
        np.stack([pack_x(xh), pack_x(xl)], axis=3)
    ).reshape(B, NCH * 128, 2 * DT * CH)
    i = np.arange(128)
    mask = np.where(i[:, None] <= i[None, :], 0.0, NEG).astype(np.float32)

    in_maps = []
    for c in range(N_CORES):
        r0 = c * NH * HD
        r1 = r0 + NH * HD
        wq = W_qkv[r0:r1].T                                   # [D, 256]
        wk = W_qkv[D + r0:D + r1].T
        wv = W_qkv[2 * D + r0:2 * D + r1].T
        wqkvT = np.concatenate([wq, wk, wv], axis=1) * 256.0  # [D, 768] f32
        wh = wqkvT.astype(E4)
        wl = (wqkvT - wh.astype(np.float32)).astype(E4)
        def pack_w(a):
            return a.reshape(DP, 2, 128, 3 * NH * HD).transpose(2, 0, 1, 3)
        w8 = np.ascontiguousarray(
            np.stack([pack_w(wh), pack_w(wl)], axis=1)).reshape(128, -1)
        woT = np.ascontiguousarray(W_out[:, r0:r1].T) / 256.0  # [256, D]
        in_maps.append({"x8": x8, "wqkvT": w8, "woT": woT, "mask": mask})
    return in_maps


def run(x, W_qkv, W_out, trace=False):
    nc = build_bass()
    in_maps = shard_inputs(x, W_qkv, W_out)
    res = run_bass_kernel_spmd(nc, in_maps, list(range(N_CORES)), trace=trace)
    parts = np.stack([r["y"].astype(np.float32) for r in res.results])
    y = parts.sum(axis=0)
    return y.reshape(B, T, D), res


def kernel(x, W_qkv, W_out):
    y, _ = run(x, W_qkv, W_out, trace=False)
    return y



# revision 53
# speedup vs baseline: 1.1954x; 1.1954x over previous
"""Causal self-attention (B=2, T=2048, D=2048, 16 heads) on 8 NeuronCores.

Tensor-parallel over heads: core c owns heads {2c, 2c+1}. Each core computes
its heads' Q/K/V projections, causal attention, and a partial output
projection (row-parallel); the host sums the 8 partials.

Layout strategy:
  - QKV runs as fp8 split3 DoubleRow chains (hi*hi + lo*hi + hi*lo) over
    512-token chunks (N=512 for q/k groups), weights pre-scaled on host
    (q/k x256, v x32).
  - Q^T/K^T/V are stored fp16 (full PE rate, half DVE cost, no fp32r
    small-N penalty). Attention works on S^T [key, query] tiles two
    k-tiles at a time; a [128,128] triangle mask is added on diagonal
    tiles; exp runs on ACT with the 1/sqrt(d)/65536 scale folded in and
    writes fp16 probabilities; PV accumulates out^T [head_dim, T] in PSUM
    with diagonal tiles trimmed to the valid column range.
  - The softmax denominator accumulates as one wide fp16 DVE add per pair
    (2x mode) into a double accumulator headed on GpSimd; two accumulating
    M=1 ones-matmuls fold it and a K=1 ones-matmul broadcasts the
    reciprocal back. Normalized out^T is split on-chip into fp8 hi+lo.
  - The output projection is fp8 DoubleRow split3 (heads are the two DR
    rows), 3 matmuls per [128,512] tile; the 2^-10 result scale from the
    v/wo pre-scaling is folded into the y copy.
  - Issue order is software-pipelined: S(p+1) and out-proj filler tiles
    are issued between S(p) and PV(p) so PE never waits on ACT's exp; the
    next x chunk's DMA is prefetched at attention start.
"""
import itertools
import numpy as np
import ml_dtypes
from collections import deque
from contextlib import ExitStack

import concourse.bass as bass
import concourse.tile as tile
from concourse import bacc
from concourse import mybir
from concourse.bass_utils import run_bass_kernel_spmd

f32 = mybir.dt.float32
f32r = mybir.dt.float32r
f8 = mybir.dt.float8e4
f16 = mybir.dt.float16
E4 = ml_dtypes.float8_e4m3
DR = mybir.MatmulPerfMode.DoubleRow

B, T, D = 2, 2048, 2048
H, HD = 16, 128
N_CORES = 8
NH = H // N_CORES            # heads per core = 2
SCALE = float(HD) ** -0.5    # 1/sqrt(128)
NEG = -1.0e9

DT = D // 128                # 16 D-tiles (contraction)
DP = DT // 2                 # 8 DoubleRow di-pairs
CH = 512                     # token chunk for QKV projection
NCH = T // CH                # 4 chunks per batch
TT = T // 128                # 16 token tiles per batch
QB = 512                     # query block for attention / feature block
NQB = T // QB                # 4
QK_SCALE = 256.0             # host prescale for q/k weights
V_SCALE = 32.0               # host prescale for v weights (fp8-safe range)
WO_SCALE = 32.0              # host prescale for wo (fp8-safe range)
Y_SCALE = 1.0 / (V_SCALE * WO_SCALE)


def _body(ctx, tc, x8, wqkvT, wo8, mask, y):
    nc = tc.nc

    singles = ctx.enter_context(tc.tile_pool(name="singles", bufs=1))
    wqkv_sb = singles.tile([128, 2, DP, 2, 3 * NH * HD], f8)
    wqkvT_r = wqkvT.rearrange("p (s a i c) -> p s a i c", s=2, a=DP, i=2)
    # wo8/mask are loaded later (first needed at attention/out-proj time).
    wo8_sb = singles.tile([128, 2, NH, D], f8)
    mask_sb = singles.tile([128, 128], f32)
    # Memset doesn't support f16/f32r; stage in f32 and convert via DVE copy.
    ones_col_f = singles.tile([128, 1], f32)
    nc.vector.memset(ones_col_f, 1.0)
    ones_col = singles.tile([128, 1], f16)
    nc.vector.tensor_copy(ones_col, ones_col_f)
    ones_row_f = singles.tile([1, 128], f32)
    nc.vector.memset(ones_row_f, 1.0)
    ones_row = singles.tile([1, 128], f32r)
    nc.vector.tensor_copy(ones_row, ones_row_f)
    zeros_f = singles.tile([128, 128], f32)
    nc.vector.memset(zeros_f, 0.0)
    zeros_sb = singles.tile([128, 128], f16)
    nc.vector.tensor_copy(zeros_sb, zeros_f)
    # Dummy activation: forces the ACT function-table load during the
    # startup DMA window instead of stalling the first real exp.
    act_warm = singles.tile([128, 1], f32)
    nc.scalar.activation(act_warm, ones_col_f,
                         mybir.ActivationFunctionType.Exp, scale=1.0)

    perbatch = ctx.enter_context(tc.tile_pool(name="perbatch", bufs=1))
    xpool = ctx.enter_context(tc.tile_pool(name="xpool", bufs=2))
    attsb = ctx.enter_context(tc.tile_pool(name="attsb", bufs=6))
    densb = ctx.enter_context(tc.tile_pool(name="densb", bufs=2))
    smallsb = ctx.enter_context(tc.tile_pool(name="smallsb", bufs=2))
    oslsb = ctx.enter_context(tc.tile_pool(name="oslsb", bufs=2))
    ysb_pool = ctx.enter_context(tc.tile_pool(name="ysb", bufs=3))
    # PSUM budget: A{qkv acc, y} 1 bank x3 + B{s2 per-j} 1 bank x3 +
    # O{attention out} 1 bank x1 + C{den, rb} 1 bank x1 = 8 banks.
    ps = ctx.enter_context(tc.tile_pool(name="ps", bufs=3, space="PSUM"))

    # Per-batch persistent tiles (tags reuse the same buffers each batch).
    def batch_tiles():
        qt_sb = perbatch.tile([128, NH, T], f16, tag="qt")     # Q^T per head
        kt_sb = perbatch.tile([128, NH, T], f16, tag="kt")     # K^T per head
        v_sb = perbatch.tile([128, NH, TT, HD], f16, tag="v")  # V per head
        o8_hi = perbatch.tile([128, NH, T], f8, tag="ohi")     # fp8 out^T hi
        o8_lo = perbatch.tile([128, NH, T], f8, tag="olo")     # fp8 out^T lo
        return qt_sb, kt_sb, v_sb, o8_hi, o8_lo

    state = {}

    def prefetch_x(b, ci, first=False):
        """Issue the DMA for batch b chunk ci into a fresh xch tile."""
        xch = xpool.tile([128, 2, DT, CH], f8, tag="xch")
        x8_b = x8[b].rearrange("(c p) f -> p c f", p=128)
        src = x8_b[:, ci, :].rearrange("p (s d t) -> p s d t", s=2, d=DT)
        if first:
            # Critical first loads: hi x + hi q/k weights first (in dp
            # halves) so the first accumulation group starts early; the
            # tiny attention mask rides along before the lo halves.
            nc.sync.dma_start(out=xch[:, 0, 0:8, :], in_=src[:, 0, 0:8, :])
            nc.sync.dma_start(out=wqkv_sb[:, 0, 0:4, :, :],
                              in_=wqkvT_r[:, 0, 0:4, :, :])
            nc.sync.dma_start(out=xch[:, 0, 8:16, :], in_=src[:, 0, 8:16, :])
            nc.sync.dma_start(out=wqkv_sb[:, 0, 4:8, :, :],
                              in_=wqkvT_r[:, 0, 4:8, :, :])
            nc.sync.dma_start(out=mask_sb, in_=mask[:, :])
            nc.sync.dma_start(out=xch[:, 1, :, :], in_=src[:, 1, :, :])
            nc.sync.dma_start(out=wqkv_sb[:, 1, :, :, :],
                              in_=wqkvT_r[:, 1, :, :, :])
        else:
            nc.sync.dma_start(out=xch[:, 0, :, :], in_=src[:, 0, :, :])
            nc.sync.dma_start(out=xch[:, 1, :, :], in_=src[:, 1, :, :])
        state[(b, ci)] = xch

    def qkv_chunk(b, ci, qt_sb, kt_sb, v_sb, fillers=None):
        """QKV projection for one 512-token chunk: four N=512 q/k groups
        and four N=256 v token-tile groups, each a split3 DR chain.
        Out-proj filler tiles interleave between groups — their y copies
        run on the here-idle ACT/DVE engines."""
        xch = state.pop((b, ci))

        def filler():
            if fillers:
                fillers.popleft()("act")
        # split3: hi*hi + lo*hi + hi*lo DoubleRow chains (drop lo*lo)
        chains = ((0, 0), (1, 0), (0, 1))
        cols = slice(ci * CH, (ci + 1) * CH)
        for kind, idx in (("q", 0), ("q", 1), ("k", 0), ("k", 1)):
            acc = ps.tile([128, CH], f32, tag="A")
            base = idx if kind == "q" else NH + idx
            mm, n_mm = 0, len(chains) * DP
            for xs, ws in chains:
                for dp in range(DP):
                    nc.tensor.matmul(
                        acc,
                        lhsT=wqkv_sb[:, ws, dp, :, base * HD:(base + 1) * HD],
                        rhs=xch[:, xs, 2 * dp:2 * dp + 2, :],
                        start=(mm == 0), stop=(mm == n_mm - 1), perf_mode=DR)
                    mm += 1
            if kind == "q":
                nc.vector.tensor_copy(qt_sb[:, idx, cols], acc)
            else:
                nc.scalar.copy(kt_sb[:, idx, cols], acc)
            filler()
        for tt in range(CH // 128):
            acc = ps.tile([128, NH * HD], f32, tag="A")
            mm, n_mm = 0, len(chains) * DP
            for xs, ws in chains:
                for dp in range(DP):
                    nc.tensor.matmul(
                        acc,
                        lhsT=xch[:, xs, 2 * dp:2 * dp + 2,
                                 tt * 128:(tt + 1) * 128],
                        rhs=wqkv_sb[:, ws, dp, :, 2 * NH * HD:3 * NH * HD],
                        start=(mm == 0), stop=(mm == n_mm - 1), perf_mode=DR)
                    mm += 1
            nc.vector.tensor_copy(
                v_sb[:, :, ci * (CH // 128) + tt, :],
                acc.rearrange("p (h d) -> p h d", h=NH))
            filler()
            filler()

    def op_tiles(b, qb, h, o8_hi, o8_lo, y, alt_tag=False):
        """Out-proj fp8 DR split3 tiles for query block qb, head-half h:
        yields one closure per (ti, eb) tile. alt_tag alternates the PSUM
        bank tag A/O for drain phases where no attention interleaves."""
        t0 = qb * (QB // 128)
        cell = {}
        for n, (ti, eb) in enumerate(
                (ti, eb)
                for ti in range(t0 + 2 * h, t0 + 2 * h + 2)
                for eb in range(D // QB)):
            def one(eng="dve", ti=ti, eb=eb, n=n):
                if alt_tag and n % 3 == 2:
                    y_ps = ps.tile([128, QB], f32, tag="O", bufs=1)
                else:
                    y_ps = ps.tile([128, QB], f32, tag="A")
                lhi = o8_hi[:, :, ti * 128:(ti + 1) * 128]
                llo = o8_lo[:, :, ti * 128:(ti + 1) * 128]
                esl = slice(eb * QB, (eb + 1) * QB)
                nc.tensor.matmul(y_ps, lhsT=lhi, rhs=wo8_sb[:, 0, :, esl],
                                 start=True, stop=False, perf_mode=DR)
                nc.tensor.matmul(y_ps, lhsT=llo, rhs=wo8_sb[:, 0, :, esl],
                                 start=False, stop=False, perf_mode=DR)
                nc.tensor.matmul(y_ps, lhsT=lhi, rhs=wo8_sb[:, 1, :, esl],
                                 start=False, stop=True, perf_mode=DR)
                # One [128, D] row tile per ti; single DMA per row keeps
                # the serialized HWDGE issue count low.
                if eb == 0:
                    cell[ti] = ysb_pool.tile([128, D], f16, tag="yt",
                                             name="y_row")
                y_row = cell[ti]
                # Copy engine picked by caller context: DVE inside the
                # ACT-saturated attention region, alternating in QKV.
                if eng == "dve" or (eng == "act" and n % 2 == 0):
                    nc.vector.tensor_scalar_mul(y_row[:, esl], y_ps, Y_SCALE)
                else:
                    nc.scalar.activation(
                        y_row[:, esl], y_ps,
                        mybir.ActivationFunctionType.Copy, scale=Y_SCALE)
                rows = slice(b * T + ti * 128, b * T + (ti + 1) * 128)
                if alt_tag:
                    # Drain phase: half-row DMAs so the final transfer
                    # overlaps the remaining copies.
                    if eb == 1:
                        nc.sync.dma_start(out=y[rows, 0:D // 2],
                                          in_=y_row[:, 0:D // 2])
                    elif eb == 3:
                        nc.sync.dma_start(out=y[rows, D // 2:],
                                          in_=y_row[:, D // 2:])
                        del cell[ti]
                elif eb == D // QB - 1:
                    nc.sync.dma_start(out=y[rows, :], in_=y_row)
                    del cell[ti]
            yield one

    def attention(qb, h, qt_sb, kt_sb, v_sb, o8_hi, o8_lo, fillers):
        nk = (qb + 1) * QB // 128
        npairs = nk // 2
        o_ps = ps.tile([128, QB], f32, tag="O", bufs=1)
        f_eng = (itertools.cycle(("dve", "act")) if len(fillers) > 4
                 else itertools.cycle(("dve", "dve", "act")))
        # Double accumulator: one wide fp16 DVE add per k-tile pair; the
        # two halves are folded by the PE ones-matmul reduction.
        den2 = densb.tile([128, 2, QB], f16, tag="den")
        q_slice = qt_sb[:, h, qb * QB:(qb + 1) * QB]
        pts = {}

        def rel(p, j):
            k_rel0 = (2 * p) * 128 - qb * QB
            diag = k_rel0 >= 0
            kr = max(k_rel0 + j * 128, 0) if diag else 0
            return kr, diag

        def s_j(p, j):
            # Per-j s2/pt tiles: finer PSUM recycling (3x1-bank rotation)
            # and earlier exp completion for PV j=0.
            kt = 2 * p + j
            kr, diag = rel(p, j)
            s2 = ps.tile([128, QB], f32, tag="B")
            pt = attsb.tile([128, QB], f16, tag="pt")
            nc.tensor.matmul(
                s2[:, kr:],
                lhsT=kt_sb[:, h, kt * 128:(kt + 1) * 128],
                rhs=q_slice[:, kr:], start=True, stop=True)
            if diag:
                # Diagonal tile: triangle mask, then exp restricted to
                # the valid column range. Columns below the diagonal are
                # never read downstream (PV and den are restricted the
                # same way), so no memset needed.
                nc.vector.tensor_add(
                    s2[:, kr:kr + 128], s2[:, kr:kr + 128], mask_sb)
            nc.scalar.activation(
                pt[:, kr:], s2[:, kr:],
                mybir.ActivationFunctionType.Exp, scale=SCALE / 65536.0)
            pts[(p, j)] = pt

        def pv_j(p, j):
            pt = pts.pop((p, j))
            kt = 2 * p + j
            kr, diag = rel(p, j)
            nc.tensor.matmul(
                o_ps[:, kr:], lhsT=v_sb[:, h, kt, :], rhs=pt[:, kr:],
                start=(kt == 0), stop=(kt == nk - 1))
            if p == 0:
                if diag and j == 1:
                    # qb == 0: j=1 starts at column 128 — zero-fill the
                    # gap so the PE fold below reads initialized data.
                    nc.gpsimd.tensor_copy(den2[:, 1, 128:], pt[:, 128:])
                    nc.vector.tensor_copy(den2[:, 1, 0:128], zeros_sb)
                else:
                    # 1-input copy runs near line-rate on GpSimd, keeping
                    # the chain head off the busy DVE.
                    nc.gpsimd.tensor_copy(den2[:, j, :], pt)
            else:
                nc.vector.tensor_add(den2[:, j, kr:], den2[:, j, kr:],
                                     pt[:, kr:])

        def filler(n=1):
            for _ in range(n):
                if fillers:
                    fillers.popleft()(next(f_eng))

        # Depth-2 software pipeline: PV(p) issues after S(p+2), giving
        # exp(p) ~3 matmul slots of PE time to complete — no PE stall even
        # with no fillers left. S(p) j0/j1 straddle PV(p-2) j0 so the s2
        # ring (3 banks) never backs up.
        s_j(0, 0)
        s_j(0, 1)
        if npairs > 1:
            s_j(1, 0)
            filler()
            s_j(1, 1)
        for p in range(2, npairs):
            s_j(p, 0)
            filler()
            pv_j(p - 2, 0)
            s_j(p, 1)
            pv_j(p - 2, 1)
        if npairs > 1:
            filler()
            pv_j(npairs - 2, 0)
            filler()
            pv_j(npairs - 2, 1)
        filler()
        pv_j(npairs - 1, 0)
        filler()
        pv_j(npairs - 1, 1)

        # Softmax denominator fold + reciprocal broadcast + fp8 split.
        den_ps = ps.tile([1, QB], f32, tag="C", bufs=1)
        nc.tensor.matmul(den_ps, lhsT=ones_col, rhs=den2[:, 0, :],
                         start=True, stop=False)
        filler()
        nc.tensor.matmul(den_ps, lhsT=ones_col, rhs=den2[:, 1, :],
                         start=False, stop=True)
        filler(3)
        recip = smallsb.tile([1, QB], f32r, tag="rcp")
        nc.vector.reciprocal(recip, den_ps)
        rb_ps = ps.tile([128, QB], f32, tag="C", bufs=1)
        nc.tensor.matmul(rb_ps, lhsT=ones_row, rhs=recip,
                         start=True, stop=True)
        filler(2)
        # DVE may read only one PSUM operand per op: copy o_ps to SBUF
        # first, then scale in place against the PSUM broadcast.
        osl = oslsb.tile([128, QB], f32r, tag="osl")
        nc.scalar.copy(osl, o_ps)
        nc.vector.tensor_mul(osl, osl, rb_ps)
        ohsl = o8_hi[:, h, qb * QB:(qb + 1) * QB]
        nc.scalar.copy(ohsl, osl)
        nc.vector.tensor_sub(o8_lo[:, h, qb * QB:(qb + 1) * QB], osl, ohsl)

    # Interleave: attention for query block c needs only the first c+1
    # QKV chunks, so QKV (pure PE) overlaps attention's DVE/ACT load; the
    # output projection lags one block and its tiles fill the exp-latency
    # bubbles inside the attention k-loop. The next x chunk's DMA is
    # prefetched at attention start so QKV never waits on HBM.
    # Out-proj tiles are split between two consumers: a few fill PE bubbles
    # inside the ACT-bound attention loops (y copies on DVE); the rest
    # interleave between QKV groups where ACT/DVE sit idle.
    ATT_SHARE = 3
    att_fill = deque()
    qkv_fill = deque()
    for b in range(B):
        qt_sb, kt_sb, v_sb, o8_hi, o8_lo = batch_tiles()
        for c in range(NQB):
            if b == 0 and c == 0:
                prefetch_x(0, 0, first=True)
            qkv_chunk(b, c, qt_sb, kt_sb, v_sb, qkv_fill)
            if b > 0 and c == 0:
                # Previous batch's leftover out-proj tiles read the prior
                # o8 tile generation; they must all issue before this
                # batch's first attention finishes (fillers issue during
                # its k-loop, safely before the o8 rewrite in its tail).
                # Keep 6 as fillers for the otherwise-empty att(0,*).
                att_fill.extend(qkv_fill)
                qkv_fill.clear()
                eng = itertools.cycle(("dve", "act"))
                while len(att_fill) > 6:
                    att_fill.popleft()(next(eng))
            if b == 0 and c == 1:
                nc.sync.dma_start(
                    out=wo8_sb,
                    in_=wo8.rearrange("p (s n e) -> p s n e", s=2, n=NH))
            # Prefetch the next chunk (possibly next batch's first).
            if c + 1 < NQB:
                prefetch_x(b, c + 1)
            elif b + 1 < B:
                prefetch_x(b + 1, 0)
            for h in range(NH):
                if c > 0:
                    tiles = list(op_tiles(b, c - 1, h, o8_hi, o8_lo, y))
                    att_fill.extend(tiles[:ATT_SHARE])
                    qkv_fill.extend(tiles[ATT_SHARE:])
                attention(c, h, qt_sb, kt_sb, v_sb, o8_hi, o8_lo, att_fill)
        # Final block's out-proj: defer into the next batch's QKV chunks;
        # on the last batch drain immediately with the A/A/O PSUM-tag
        # rotation (o_ps is free) to hide the y-copy latency.
        if b + 1 < B:
            for h in range(NH):
                qkv_fill.extend(op_tiles(b, NQB - 1, h, o8_hi, o8_lo, y))
        else:
            for dq in (att_fill, qkv_fill):
                while dq:
                    dq.popleft()("act")
            for h in range(NH):
                for f in op_tiles(b, NQB - 1, h, o8_hi, o8_lo, y,
                                  alt_tag=True):
                    f("act")


_NC_CACHE = {}


def build_bass(do_compile=True):
    if do_compile in _NC_CACHE:
        return _NC_CACHE[do_compile]
    nc = bacc.Bacc()
    x8 = nc.declare_dram_parameter("x8", [B, NCH * 128, 2 * DT * CH], f8,
                                   isOutput=False)
    wqkvT = nc.declare_dram_parameter("wqkvT", [128, 2 * DP * 2 * 3 * NH * HD],
                                      f8, isOutput=False)
    wo8 = nc.declare_dram_parameter("wo8", [128, 2 * NH * D], f8,
                                    isOutput=False)
    mask = nc.declare_dram_parameter("mask", [128, 128], f32, isOutput=False)
    y = nc.declare_dram_parameter("y", [B * T, D], f16, isOutput=True)
    with tile.TileContext(nc) as tc:
        with ExitStack() as ctx:
            with nc.allow_low_precision(
                    reason="fp8/fp16 tiles feed full-rate PE matmuls; the "
                           "fp8 paths carry hi+lo split correction terms"):
                _body(ctx, tc, x8, wqkvT, wo8, mask, y[:, :])
    if do_compile:
        nc.compile()
    _NC_CACHE[do_compile] = nc
    return nc


def shard_inputs(x, W_qkv, W_out):
    x = np.asarray(x, dtype=np.float32)
    W_qkv = np.asarray(W_qkv, dtype=np.float32)
    W_out = np.asarray(W_out, dtype=np.float32)

    xh = x.astype(E4)
    xl = (x - xh.astype(np.float32)).astype(E4)
    # packed per-chunk layout: [B, NCH, 128p, hi/lo, DT, CH]
    def pack_x(a):
        return a.reshape(B, NCH, CH, DT, 128).transpose(0, 1, 4, 3, 2)
    x8 = np.ascontiguousarray(
        np.stack([pack_x(xh), pack_x(xl)], axis=3)
    ).reshape(B, NCH * 128, 2 * DT * CH)
    i = np.arange(128)
    mask = np.where(i[:, None] <= i[None, :], 0.0, NEG).astype(np.float32)

    in_maps = []
    for c in range(N_CORES):
        r0 = c * NH * HD
        r1 = r0 + NH * HD
        wq = W_qkv[r0:r1].T * QK_SCALE                         # [D, 256]
        wk = W_qkv[D + r0:D + r1].T * QK_SCALE
        wv = W_qkv[2 * D + r0:2 * D + r1].T * V_SCALE
        wqkvT = np.concatenate([wq, wk, wv], axis=1)           # [D, 768] f32
        wh = wqkvT.astype(E4)
        wl = (wqkvT - wh.astype(np.float32)).astype(E4)
        def pack_w(a):
            return a.reshape(DP, 2, 128, 3 * NH * HD).transpose(2, 0, 1, 3)
        w8 = np.ascontiguousarray(
            np.stack([pack_w(wh), pack_w(wl)], axis=1)).reshape(128, -1)
        wo = np.ascontiguousarray(W_out[:, r0:r1].T) * WO_SCALE  # [256, D]
        woh = wo.astype(E4)
        wol = (wo - woh.astype(np.float32)).astype(E4)
        # [128p(hd), hi/lo, NH, D]
        def pack_wo(a):
            return a.reshape(NH, 128, D)
        wo8 = np.ascontiguousarray(np.stack(
            [pack_wo(woh), pack_wo(wol)], axis=0).transpose(2, 0, 1, 3)
        ).reshape(128, 2 * NH * D)
        in_maps.append({"x8": x8, "wqkvT": w8, "wo8": wo8, "mask": mask})
    return in_maps


def run(x, W_qkv, W_out, trace=False):
    nc = build_bass()
    in_maps = shard_inputs(x, W_qkv, W_out)
    res = run_bass_kernel_spmd(nc, in_maps, list(range(N_CORES)), trace=trace)
    parts = np.stack([r["y"].astype(np.float32) for r in res.results])
    y = parts.sum(axis=0)
    return y.reshape(B, T, D), res


def kernel(x, W_qkv, W_out):
    y, _ = run(x, W_qkv, W_out, trace=False)
    return y
